# revision 1
# baseline (speedup 1.0000x reference)
"""CQT magnitude kernel for Trainium2 (8 NeuronCores, Bass/Tile).

Strategy
--------
The CQT C[k, n] = sum_l xpad[n*HOP + l] * kernel[k, l] is regrouped over
128-wide l-chunks: with X128[p, j] = xpad[j*128 + p] (the signal transposed
into a [128, cols] SBUF-resident layout) and HOP = 512 = 4*128,

    C[k, n] = sum_c sum_p kernel[k, c*128 + p] * X128[p, c + 4n]

i.e. for every l-chunk c one PE matmul:  lhsT = kernelT[c] ([128 l, bins]),
rhs = strided view of X128 ([128 l, (frame, track) columns]), accumulated in
PSUM over c.  The filterbank rows 128..251 are zero outside a small center
window (constant-Q support shrinks with frequency), so those bins only get
matmuls for the 48 center chunks.

Sharding: the contraction (l-chunk) axis is split 8 ways.  Each core gets a
pre-transposed, pre-packed slice of the filterbank (host-side numpy packing
keeps the program SPMD-uniform: all per-core variation lives in the input
tensors), computes partial re/im sums for ALL bins/tracks/frames, and the
host sums the 8 partials and takes sqrt(re^2 + im^2).

Numerics: operands are cast to bf16 on the host (PE streams bf16 at 1
cycle/row vs 2 for f32/f32r — measured), accumulation is f32 in PSUM.
End-to-end relative error ~2.5e-3.
"""

import numpy as np

# ---- problem constants (hardcoded per contract) ----
SR_B, SR_TR, SR_T = 2, 2, 65536        # x shape
NTRACKS = SR_B * SR_TR                 # 4
KBINS = 252
L = 69376                              # filterbank window length
HOP = 512
PCH = 128                              # l-chunk width = contraction size
NCH = L // PCH                         # 542 l-chunks
NF = 1 + SR_T // HOP                   # 129 frames
N2 = 2 * NF                            # 258 matmul columns (2 tracks)
NCORES = 8

M1_C0 = 247                            # first nonzero l-chunk for bins 128..251
M1_NCH = 48                            # number of such chunks (validated in proto)
PER0 = 544 // NCORES                   # 68 m0 slots per core (542 padded to 544)
PER1 = M1_NCH // NCORES                # 6 m1 slots per core
SLOTS = PER0 + PER1                    # 74

M0_COLS = PER0 + 4 * (NF - 1)          # 580 xi columns for the m0 block
M1_COLS = PER1 + 4 * (NF - 1)          # 518 xi columns for the m1 block
XI_COLS = M0_COLS + M1_COLS            # 1098
XPAD_COLS = 1056                       # padded signal columns (134912/128 = 1054, +2 pad)

NGROUPS = 10                           # kt DMA groups: 8 slots apiece (last has 2)
GSIZES = [8] * 9 + [2]
NWARM = 28                             # PE pre-warm matmuls (HAM unthrottle ~3.4us)

_PROG = None


def _build_program():
    import concourse.bass as bass
    import concourse.mybir as mybir
    from concourse import bacc
    from concourse.tile import TileContext

    f32 = mybir.dt.float32
    bf16 = mybir.dt.bfloat16

    nc = bacc.Bacc(None, name="cqt_spmd")
    kt_d = nc.dram_tensor("kt", [NGROUPS, 128, 8 * 256], bf16, kind="ExternalInput")
    xi_d = nc.dram_tensor("xi", [128, XI_COLS, 4], bf16, kind="ExternalInput")
    out_d = nc.dram_tensor("out", [8, 128, N2], f32, kind="ExternalOutput")

    with TileContext(nc) as tc:
        with (
            tc.tile_pool(name="xip", bufs=1) as xip,
            tc.tile_pool(name="ktp", bufs=NGROUPS) as ktp,
            tc.tile_pool(name="wp", bufs=1) as wp,
            tc.tile_pool(name="accp", bufs=1, space="PSUM") as accp,
        ):
            xi_t = xip.tile([128, XI_COLS, 4], bf16)
            nc.gpsimd.dma_start(out=xi_t, in_=xi_d[:, :, :])

            accs = [
                accp.tile([128, N2], f32, tag=f"acc{b}", name=f"acc{b}")
                for b in range(8)
            ]

            # PE pre-warm: keep the PE busy from t~0 so the HAM clock gate
            # opens (1.2 -> 2.4 GHz) before the real matmul stream begins.
            # Writes land in acc bank 7, which is re-initialized (start=True)
            # by its first real matmul later.
            wtile = wp.tile([128, 128], bf16)
            nc.vector.memset(wtile, 0.0)
            for _ in range(NWARM):
                nc.tensor.matmul(
                    accs[7][:, :128], wtile, wtile, start=True, stop=True
                )

            for g in range(NGROUPS):
                gs = GSIZES[g]
                kt_t = ktp.tile([128, 8 * 256], bf16, tag="kt")
                nc.gpsimd.dma_start(
                    out=kt_t[:, : gs * 256], in_=kt_d[g, :, : gs * 256]
                )
                for si in range(gs):
                    s = 8 * g + si
                    m = 0 if s < PER0 else 1
                    first = s == 0 or s == PER0
                    last = s == PER0 - 1 or s == SLOTS - 1
                    for part in range(2):
                        j0 = (si * 2 + part) * 128
                        lhsT = kt_t[:, j0 : j0 + 128]
                        for tp in range(2):
                            base = (
                                s * 4 if m == 0 else M0_COLS * 4 + (s - PER0) * 4
                            ) + tp * 2
                            rhs = bass.AP(
                                tensor=xi_t.tensor,
                                offset=xi_t.offset + base,
                                ap=[xi_t.ap[0], [16, NF], [1, 2]],
                            )
                            nc.tensor.matmul(
                                accs[m * 4 + part * 2 + tp],
                                lhsT,
                                rhs,
                                start=first,
                                stop=last,
                            )

            # flush accumulators per-bank (PSUM -> SBUF via DVE, then DMA);
            # the four m0 banks can flush while m1 matmuls still run
            for b in range(8):
                st = wp.tile([128, N2], f32, tag=f"st{b}", name=f"st{b}")
                nc.vector.tensor_copy(st, accs[b])
                nc.gpsimd.dma_start(out=out_d[b, :, :], in_=st)
    nc.finalize()  # Bacc: runs compile() (reg alloc, event-sem legalization)
    return nc


def _pack_inputs(x, kr, ki):
    import ml_dtypes

    bf16 = ml_dtypes.bfloat16
    xf = np.ascontiguousarray(np.asarray(x, dtype=np.float32).reshape(NTRACKS, SR_T))
    kr = np.asarray(kr, dtype=np.float32)
    ki = np.asarray(ki, dtype=np.float32)

    # transposed filterbank halves, zero-padded to 128 bins each
    krT0 = np.ascontiguousarray(kr[:128].T)          # [L, 128]
    kiT0 = np.ascontiguousarray(ki[:128].T)

    def padT(mat):
        buf = np.zeros((128, L), np.float32)
        buf[: mat.shape[0]] = mat
        return np.ascontiguousarray(buf.T)

    krT1 = padT(kr[128:])
    kiT1 = padT(ki[128:])

    # signal, padded and transposed: XI_full[p, j, t] = xpad[t, j*128 + p]
    xpad = np.zeros((NTRACKS, XPAD_COLS * PCH), np.float32)
    xpad[:, L // 2 : L // 2 + SR_T] = xf
    XI_full = np.ascontiguousarray(
        xpad.reshape(NTRACKS, XPAD_COLS, PCH).transpose(2, 1, 0)
    )  # [128, 1056, 4]

    in_maps = []
    for q in range(NCORES):
        c0 = q * PER0
        c1 = M1_C0 + q * PER1
        kt_all = np.zeros((80, 2, PCH, 128), np.float32)
        n_real = min(PER0, NCH - c0)
        kt_all[:n_real, 0] = krT0[c0 * 128 : (c0 + n_real) * 128].reshape(
            n_real, 128, 128
        )
        kt_all[:n_real, 1] = kiT0[c0 * 128 : (c0 + n_real) * 128].reshape(
            n_real, 128, 128
        )
        kt_all[PER0:SLOTS, 0] = krT1[c1 * 128 : (c1 + PER1) * 128].reshape(
            PER1, 128, 128
        )
        kt_all[PER0:SLOTS, 1] = kiT1[c1 * 128 : (c1 + PER1) * 128].reshape(
            PER1, 128, 128
        )
        ktg = np.ascontiguousarray(
            kt_all.reshape(NGROUPS, 8, 2, 128, 128)
            .transpose(0, 3, 1, 2, 4)
            .reshape(NGROUPS, 128, 2048)
            .astype(bf16)
        )
        xi = np.ascontiguousarray(
            np.concatenate(
                [XI_full[:, c0 : c0 + M0_COLS], XI_full[:, c1 : c1 + M1_COLS]],
                axis=1,
            ).astype(bf16)
        )
        in_maps.append({"kt": ktg, "xi": xi})
    return in_maps


def _combine(outs):
    re_acc = np.zeros((KBINS, NTRACKS, NF), np.float32)
    im_acc = np.zeros((KBINS, NTRACKS, NF), np.float32)
    for q in range(NCORES):
        out = outs[q]  # [8, 128, 258]
        for b in range(8):
            m, part, tp = b >> 2, (b >> 1) & 1, b & 1
            arr = out[b].reshape(128, NF, 2)
            rows = slice(0, 128) if m == 0 else slice(128, KBINS)
            nrows = 128 if m == 0 else KBINS - 128
            tgt = re_acc if part == 0 else im_acc
            tgt[rows, tp * 2 : (tp + 1) * 2] += arr[:nrows].transpose(0, 2, 1)
    y = np.sqrt(re_acc**2 + im_acc**2)  # [252, 4, 129]
    return np.ascontiguousarray(
        y.reshape(KBINS, SR_B, SR_TR, NF).transpose(1, 0, 3, 2)
    )


def kernel(x, kr, ki):
    global _PROG
    from concourse.bass_utils import run_bass_kernel_spmd

    if _PROG is None:
        _PROG = _build_program()
    in_maps = _pack_inputs(x, kr, ki)
    res = run_bass_kernel_spmd(_PROG, in_maps, core_ids=list(range(NCORES)))
    outs = [res.results[q]["out"] for q in range(NCORES)]
    return _combine(outs)



# revision 3
# speedup vs baseline: 1.9719x; 1.9719x over previous
"""CQT magnitude kernel for Trainium2 (8 NeuronCores, Bass/Tile).

Strategy (v2: symmetry-folded)
------------------------------
The CQT filterbank is exactly symmetric about its center tap: kr (Hann*cos)
is even, ki (Hann*sin) is odd.  Folding the contraction halves PE work:

    re[k,n] = sum_u kr_f[k,u] * s[n,u],   s[n,u] = x[nH+c+u] + x[nH+c-u]
    im[k,n] = sum_u ki_f[k,u] * d[n,u],   d[n,u] = x[nH+c+u] - x[nH+c-u]

with u in [0, L/2), kr_f[k,0] halved (s[n,0] = 2*x_center).  s/d are formed
on the DVE from two SBUF-resident signal layouts: Xf[p,j] = xpad[j*128+p]
(forward) and Xr[p,j] = xpad[(j+1)*128-p] (partition-reversed), so that for
u-chunk c, frame n:  x[+u] = Xf[p, 271+c+4n],  x[-u] = Xr[p, 270-c+4n].

Work: 271 m0 u-chunks (bins 0..127) + 24 m1 u-chunks (bins 128..251) + 1 pad
= 296 units, 37 per core.  Per unit: one DVE add + one sub ([128,516] bf16)
and 4 PE matmuls (re/im x 2 track-pairs, N=258) accumulating in 8 PSUM banks
(class A = slots 0..11 -> banks 0..3, class B = slots 12..36 -> banks 4..7).
Each (core, class) cell holds chunks of a single bin-block; the host maps
cell partials to bins: cores 0-5 all m0; cores 6,7 class A = m1 halves.

Flushes: Act-engine copies PSUM->SBUF (fp16), one DMA per class.  The host
sums the 16 cell partials and takes sqrt(re^2+im^2).
"""

import numpy as np

# ---- problem constants (hardcoded per contract) ----
SR_B, SR_TR, SR_T = 2, 2, 65536        # x shape
NTRACKS = SR_B * SR_TR                 # 4
KBINS = 252
L = 69376                              # filterbank window length
HL = L // 2                            # 34688 folded taps
HOP = 512
PCH = 128
NCH0 = 271                             # m0 u-chunks (bins 0..127)
NCH1 = 24                              # m1 u-chunks (bins 128..251)
NF = 1 + SR_T // HOP                   # 129 frames
N2 = 2 * NF                            # 258 matmul columns (2 tracks)
NCORES = 8

NSLOTS = 37                            # per-core units (296 = 8*37 total)
NA = 12                                # class-A slots -> PSUM banks 0-3
NB = NSLOTS - NA                       # 25 class-B slots -> banks 4-7
FCA = NA + 4 * (NF - 1)                # 524 forward cols, class A window
FCB = NB + 4 * (NF - 1)                # 537 forward cols, class B window
XOFF = [0, FCA, FCA + FCB, 2 * FCA + FCB]   # FA | FB | RA | RB col starts
XCOLS = 2 * (FCA + FCB)                # 2122

GSIZES = [2, 4, 4, 4, 4, 4, 4, 4, 4, 3]   # kt DMA groups (sum = 37)
NG = len(GSIZES)
_GOF = np.cumsum([0] + GSIZES).tolist()


def _slot_group(s):
    for g in range(NG):
        if s < _GOF[g + 1]:
            return g, s - _GOF[g]
    raise ValueError(s)


# per-core chunk assignment: (classA_start, classB_start, classA_is_m1)
def _core_assign(q):
    if q < 6:
        return 37 * q, 37 * q + 12, False
    if q == 6:
        return 0, 222, True
    return 12, 247, True                # core 7; B covers 247..270 + 1 pad


NWARM = 38                             # PE pre-warm matmuls (cover DMA fill)

_PROG = None


def _build_program():
    import concourse.bass as bass
    import concourse.mybir as mybir
    from concourse import bacc
    from concourse.tile import TileContext

    f32 = mybir.dt.float32
    f16 = mybir.dt.float16
    bf16 = mybir.dt.bfloat16
    COPY = mybir.ActivationFunctionType.Copy

    nc = bacc.Bacc(None, name="cqt_fold")
    x_d = nc.dram_tensor("x", [128, XCOLS, 4], bf16, kind="ExternalInput")
    kt_d = nc.dram_tensor("kt", [NG, 128, 4 * 256], bf16, kind="ExternalInput")
    out_d = nc.dram_tensor("out", [2, 128, 4 * N2], f16, kind="ExternalOutput")

    def vw(t, off, pat):
        return bass.AP(tensor=t.tensor, offset=t.offset + off, ap=[t.ap[0]] + pat)

    with TileContext(nc) as tc:
        with (
            tc.tile_pool(name="xp", bufs=1) as xp,
            tc.tile_pool(name="ktp", bufs=NG) as ktp,
            tc.tile_pool(name="sp", bufs=3) as sp,
            tc.tile_pool(name="dp", bufs=3) as dp,
            tc.tile_pool(name="wp", bufs=1) as wp,
            tc.tile_pool(name="accp", bufs=1, space="PSUM") as accp,
        ):
            fa_t = xp.tile([128, FCA, 4], bf16, name="fa")
            fb_t = xp.tile([128, FCB, 4], bf16, name="fb")
            ra_t = xp.tile([128, FCA, 4], bf16, name="ra")
            rb_t = xp.tile([128, FCB, 4], bf16, name="rb")

            kts = [
                ktp.tile([128, 4 * 256], bf16, tag="kt", name=f"kt{g}")
                for g in range(NG)
            ]

            # DMA issue order = transfer order: critical path first
            nc.gpsimd.dma_start(out=fa_t, in_=x_d[:, XOFF[0]:XOFF[0] + FCA, :])
            nc.gpsimd.dma_start(out=ra_t, in_=x_d[:, XOFF[2]:XOFF[2] + FCA, :])
            for g in (0, 1):
                nc.gpsimd.dma_start(
                    out=kts[g][:, : GSIZES[g] * 256],
                    in_=kt_d[g, :, : GSIZES[g] * 256],
                )
            nc.gpsimd.dma_start(out=fb_t, in_=x_d[:, XOFF[1]:XOFF[1] + FCB, :])
            nc.gpsimd.dma_start(out=rb_t, in_=x_d[:, XOFF[3]:XOFF[3] + FCB, :])
            for g in range(2, NG):
                nc.gpsimd.dma_start(
                    out=kts[g][:, : GSIZES[g] * 256],
                    in_=kt_d[g, :, : GSIZES[g] * 256],
                )

            accs = [
                accp.tile([128, N2], f32, tag=f"acc{b}", name=f"acc{b}")
                for b in range(8)
            ]
            sta = wp.tile([128, 4 * N2], f16, name="sta")
            stb = wp.tile([128, 4 * N2], f16, name="stb")

            # PE pre-warm while the input DMAs land
            wtile = wp.tile([128, 128], bf16, name="warm")
            nc.vector.memset(wtile, 0.0)
            for _ in range(NWARM):
                nc.tensor.matmul(
                    accs[7][:, :128], wtile, wtile, start=True, stop=True
                )

            FRAME_AP = [[16, NF], [1, 4]]      # frames x 4 tracks on x tiles
            for s in range(NSLOTS):
                cls = 0 if s < NA else 1
                if cls == 0:
                    f_t, r_t = fa_t, ra_t
                    f_off, r_off = 4 * s, 4 * (NA - 1 - s)
                else:
                    f_t, r_t = fb_t, rb_t
                    f_off, r_off = 4 * (s - NA), 4 * (NSLOTS - 1 - s)
                first = s == 0 or s == NA
                last = s == NA - 1 or s == NSLOTS - 1

                s_t = sp.tile([128, 4 * NF], bf16, tag="s")
                d_t = dp.tile([128, 4 * NF], bf16, tag="d")
                f_ap = vw(f_t, f_off, FRAME_AP)
                r_ap = vw(r_t, r_off, FRAME_AP)
                nc.vector.tensor_add(vw(s_t, 0, [[4, NF], [1, 4]]), f_ap, r_ap)
                nc.vector.tensor_sub(vw(d_t, 0, [[4, NF], [1, 4]]), f_ap, r_ap)

                g, si = _slot_group(s)
                for part in range(2):
                    lhsT = kts[g][:, si * 256 + part * 128: si * 256 + (part + 1) * 128]
                    src = s_t if part == 0 else d_t
                    for tp in range(2):
                        rhs = vw(src, 2 * tp, [[4, NF], [1, 2]])
                        nc.tensor.matmul(
                            accs[cls * 4 + part * 2 + tp],
                            lhsT,
                            rhs,
                            start=first,
                            stop=last,
                        )

                if s == NA - 1:
                    for b in range(4):
                        nc.scalar.activation(
                            sta[:, b * N2:(b + 1) * N2], accs[b], COPY
                        )
                    nc.gpsimd.dma_start(out=out_d[0, :, :], in_=sta)

            for b in range(4):
                nc.scalar.activation(
                    stb[:, b * N2:(b + 1) * N2], accs[4 + b], COPY
                )
            nc.gpsimd.dma_start(out=out_d[1, :, :], in_=stb)
    nc.finalize()
    return nc


def _take_cols(src, start, n):
    """src[:, start:start+n, :] with zero padding outside src's col range."""
    out = np.zeros((128, n, 4), np.float32)
    s0, s1 = max(0, start), min(src.shape[1], start + n)
    if s1 > s0:
        out[:, s0 - start:s1 - start] = src[:, s0:s1]
    return out


def _pack_inputs(x, kr, ki):
    import ml_dtypes

    bf16 = ml_dtypes.bfloat16
    xf = np.ascontiguousarray(np.asarray(x, dtype=np.float32).reshape(NTRACKS, SR_T))
    kr = np.asarray(kr, dtype=np.float32)
    ki = np.asarray(ki, dtype=np.float32)

    xpad = np.zeros((NTRACKS, HL + SR_T + HL), np.float32)
    xpad[:, HL:HL + SR_T] = xf
    A = xpad.reshape(NTRACKS, -1, PCH)            # [4, 1054, 128]
    Xf = np.ascontiguousarray(A.transpose(2, 1, 0))     # [128, 1054, 4]
    NR = 784                                       # reversed cols needed <= 783
    Xr = np.zeros((128, NR, 4), np.float32)
    Xr[0] = A[:, 1:NR + 1, 0].transpose(1, 0)      # p=0: xpad[(j+1)*128]
    Xr[1:] = np.flip(A[:, :NR, 1:], axis=2).transpose(2, 1, 0)

    krF = kr[:, HL:].copy()
    krF[:, 0] *= 0.5                               # s[n,0] double-counts center
    kiF = ki[:, HL:]

    in_maps = []
    for q in range(NCORES):
        cA0, cB0, a_m1 = _core_assign(q)
        fa = _take_cols(Xf, 271 + cA0, FCA)
        fb = _take_cols(Xf, 271 + cB0, FCB)
        ra = _take_cols(Xr, 270 - cA0 - (NA - 1), FCA)
        rb = _take_cols(Xr, 270 - cB0 - (NB - 1), FCB)
        xarr = np.ascontiguousarray(
            np.concatenate([fa, fb, ra, rb], axis=1).astype(bf16)
        )

        kt_all = np.zeros((NG, 128, 4 * 256), np.float32)
        for s in range(NSLOTS):
            m1 = a_m1 and s < NA
            c = cA0 + s if s < NA else cB0 + (s - NA)
            if c * PCH >= HL:
                continue                           # pad slot: zero weights
            rows = slice(128, KBINS) if m1 else slice(0, 128)
            g, si = _slot_group(s)
            for part, mat in ((0, krF), (1, kiF)):
                blk = mat[rows, c * PCH:(c + 1) * PCH]     # [nbins, 128]
                buf = np.zeros((128, PCH), np.float32)
                buf[: blk.shape[0]] = blk
                kt_all[g, :, si * 256 + part * 128: si * 256 + (part + 1) * 128] = buf.T
        in_maps.append({
            "x": xarr,
            "kt": np.ascontiguousarray(kt_all.astype(bf16)),
        })
    return in_maps


def _combine(outs):
    re_acc = np.zeros((KBINS, NTRACKS, NF), np.float32)
    im_acc = np.zeros((KBINS, NTRACKS, NF), np.float32)
    for q in range(NCORES):
        _, _, a_m1 = _core_assign(q)
        out = np.asarray(outs[q], dtype=np.float32)   # [2, 128, 4*258]
        for b in range(8):
            cls, part, tp = b >> 2, (b >> 1) & 1, b & 1
            m1 = a_m1 and cls == 0
            arr = out[cls, :, (b & 3) * N2:((b & 3) + 1) * N2].reshape(128, NF, 2)
            rows = slice(128, KBINS) if m1 else slice(0, 128)
            nrows = KBINS - 128 if m1 else 128
            tgt = re_acc if part == 0 else im_acc
            tgt[rows, tp * 2:(tp + 1) * 2] += arr[:nrows].transpose(0, 2, 1)
    y = np.sqrt(re_acc**2 + im_acc**2)                # [252, 4, 129]
    return np.ascontiguousarray(
        y.reshape(KBINS, SR_B, SR_TR, NF).transpose(1, 0, 3, 2)
    )


def kernel(x, kr, ki):
    global _PROG
    from concourse.bass_utils import run_bass_kernel_spmd

    if _PROG is None:
        _PROG = _build_program()
    in_maps = _pack_inputs(x, kr, ki)
    res = run_bass_kernel_spmd(_PROG, in_maps, core_ids=list(range(NCORES)))
    outs = [res.results[q]["out"] for q in range(NCORES)]
    return _combine(outs)
